# revision 1
# baseline (speedup 1.0000x reference)
"""Trainium2 Bass kernel for nn_AttentionFusion (B=8192, M=4, H=1024), 8-core data parallel.

Math (exact reformulation of the reference):
  scores[b,m,n] = conf[b,m] * (q_{4b+m} . k_{4b+n}) / sqrt(H)
                = conf[b,m] * (Y[4b+m] . X[4b+n] + alpha[4b+m] + beta[4b+n] + d)
      with Y = X G, G = (Wq/sqrt(H))^T Wk,
      alpha = X (G-bias cross term) = X ((Wq/32)^T bk), beta = X (Wk^T bq/32), d = (bq/32).bk
  wt[b,n] = sum_m softmax_n(scores)[b,m,n]      (convex weights * 4)
  Z[b]    = sum_n wt[b,n] X[4b+n]               (mean over m folds into Wc)
  out[b]  = Z[b] (Wo Wv / 4)^T + (bv Wo^T + bo)

So the TensorEngine does: Y = X G (8.6 GF/core), per-group gram S = Y X^T with a K=2
rank-1 fixup (1.1 GF), and out = Z Wc^T (2.15 GF) -- 11.8 GF/core vs 28 GF naive.
Everything runs feature-major ([feat, token]) so projections keep weights stationary,
per-partition biases live on the partition axis (ScalarE), the diagonal 4x4 score
blocks are gathered via a DRAM round-trip, and the DVE only does softmax + the
weighted combine that builds Z.
"""
import sys

if '/opt/trn_rl_repo' not in sys.path:
    sys.path.insert(0, '/opt/trn_rl_repo')

import numpy as np
import ml_dtypes

B, M, H = 8192, 4, 1024
NCORES = 8
B_CORE = B // NCORES            # 1024 batch rows per core
T_CORE = B_CORE * M             # 4096 tokens per core
T_SUPER = 1024                  # tokens per super-tile
P = 128
OC = H // P                     # 8 output chunks
HC = H // P                     # 8 contraction chunks
BF16 = ml_dtypes.bfloat16

_NC_CACHE = {}


def build_bass(n_super=T_CORE // T_SUPER):
    """Build the per-core Bass graph. n_super super-tiles of T_SUPER tokens."""
    import concourse.bass as bass
    import concourse.mybir as mybir
    import concourse.tile as tile
    from concourse import bacc

    t_core = n_super * T_SUPER
    b_core = t_core // M
    nb_tiles = b_core // P                     # softmax b-tiles (128 batch rows)
    b_super = T_SUPER // M                     # batch rows per super-tile (256)

    nc = bacc.Bacc(None, target_bir_lowering=False)
    xT = nc.dram_tensor("xT", [H, t_core], mybir.dt.bfloat16, kind="ExternalInput")
    wg = nc.dram_tensor("wg", [H, H], mybir.dt.bfloat16, kind="ExternalInput")
    wcT = nc.dram_tensor("wcT", [H, H], mybir.dt.bfloat16, kind="ExternalInput")
    aby = nc.dram_tensor("aby", [2, t_core], mybir.dt.bfloat16, kind="ExternalInput")
    abx = nc.dram_tensor("abx", [2, t_core], mybir.dt.bfloat16, kind="ExternalInput")
    bc = nc.dram_tensor("bc", [H], mybir.dt.float32, kind="ExternalInput")
    conf = nc.dram_tensor("conf", [b_core, M], mybir.dt.float32, kind="ExternalInput")
    outT = nc.dram_tensor("outT", [H, b_core], mybir.dt.float32, kind="ExternalOutput")

    FDT = mybir.dt.float32
    BDT = mybir.dt.bfloat16
    AX = mybir.AxisListType.X
    MUL = mybir.AluOpType.mult
    ADD = mybir.AluOpType.add

    from contextlib import ExitStack
    with tile.TileContext(nc) as tc:
        with ExitStack() as _es:
            wp = _es.enter_context(tc.tile_pool(name="wp", bufs=1))
            cp = _es.enter_context(tc.tile_pool(name="cp", bufs=1))
            xp = _es.enter_context(tc.tile_pool(name="xp", bufs=3))
            abp = _es.enter_context(tc.tile_pool(name="abp", bufs=n_super))
            yp = _es.enter_context(tc.tile_pool(name="yp", bufs=1))
            zp = _es.enter_context(tc.tile_pool(name="zp", bufs=2))
            gpl = _es.enter_context(tc.tile_pool(name="gp", bufs=2))
            smp = _es.enter_context(tc.tile_pool(name="smp", bufs=2))
            wrp = _es.enter_context(tc.tile_pool(name="wrp", bufs=2 * n_super))
            prp = _es.enter_context(tc.tile_pool(name="prp", bufs=4))
            osb = _es.enter_context(tc.tile_pool(name="osb", bufs=3))
            psp = _es.enter_context(tc.tile_pool(name="psp", bufs=3, space="PSUM"))
            psg = _es.enter_context(tc.tile_pool(name="psg", bufs=2, space="PSUM"))
            pso = _es.enter_context(tc.tile_pool(name="pso", bufs=3, space="PSUM"))
            drg = _es.enter_context(tc.tile_pool(name="drg", bufs=n_super, space="DRAM"))
            drw = _es.enter_context(tc.tile_pool(name="drw", bufs=2 * n_super, space="DRAM"))

            # ---- HAM warmup: dataless matmuls fill the PE during the DMA
            #      lead-in so the real Y stream starts at the warm 2.4 GHz clock ----
            wu = wp.tile([P, P], BDT, tag="warm", name="warm_sb")
            nc.vector.memset(wu[:], 1.0)
            wups = psg.tile([P, P], FDT, tag="gram_ps", name="warm_ps")
            for i in range(40):
                nc.tensor.matmul(wups[:], wu[:], wu[:],
                                 start=(i == 0), stop=(i == 39))
            wuo = wp.tile([P, P], FDT, tag="warmo", name="warm_out")
            nc.scalar.copy(wuo[:], wups[:])

            # ---- resident weights / constants (chunked loads for a short lead-in) ----
            wg_sb = wp.tile([P, HC, H], BDT, tag="wg", name="wg_sb")
            for oc in range(OC):
                nc.sync.dma_start(
                    wg_sb[:, :, oc * P:(oc + 1) * P],
                    wg[:, oc * P:(oc + 1) * P].rearrange("(c p) o -> p c o", p=P))
            wc_sb = wp.tile([P, HC, H], BDT, tag="wc", name="wc_sb")
            bc_sb = cp.tile([P, OC], FDT, tag="bc", name="bc_sb")
            nc.sync.dma_start(bc_sb[:], bc[:].rearrange("(c p) -> p c", p=P))
            conf_sb = cp.tile([P, nb_tiles, M], FDT, tag="conf", name="conf_sb")
            nc.sync.dma_start(conf_sb[:], conf[:].rearrange("(bt p) m -> p bt m", p=P))

            z_prev = None  # (zT tile, super-tile index) pending output projection

            def out_proj(zT, si):
                o_all = osb.tile([P, OC, b_super], FDT, tag="osb")
                for oc in range(OC):
                    pt = pso.tile([P, b_super], FDT, tag="outp")
                    for hc in range(HC):
                        nc.tensor.matmul(
                            pt[:], wc_sb[:, hc, oc * P:(oc + 1) * P], zT[:, hc, :],
                            start=(hc == 0), stop=(hc == HC - 1))
                    nc.scalar.add(o_all[:, oc, :], pt[:], bc_sb[:, oc:oc + 1])
                nc.sync.dma_start(
                    outT[:, si * b_super:(si + 1) * b_super]
                    .rearrange("(c p) b -> p c b", p=P), o_all[:])

            def out_proj_st(zT, si, st):
                o_st = osb.tile([P, OC, P], FDT, tag="osb_st")
                for oc in range(OC):
                    pt = pso.tile([P, P], FDT, tag="outp")
                    for hc in range(HC):
                        nc.tensor.matmul(
                            pt[:], wc_sb[:, hc, oc * P:(oc + 1) * P],
                            zT[:, hc, st * P:(st + 1) * P],
                            start=(hc == 0), stop=(hc == HC - 1))
                    nc.scalar.add(o_st[:, oc, :], pt[:], bc_sb[:, oc:oc + 1])
                bg = si * b_super + st * P
                nc.sync.dma_start(
                    outT[:, bg:bg + P].rearrange("(c p) b -> p c b", p=P), o_st[:])

            def softmax_block(s, st, gram_half):
                bt = s * 2 + st
                s_sb = smp.tile([P, 16], FDT, tag="s", name="s_sb")
                base = gram_half[:]
                for j in range(4):
                    src = bass.AP(base.tensor, base.offset + j * P * P,
                                  [[4 * P + 4, 32], [P, 4], [1, 4]])
                    nc.gpsimd.dma_start(
                        s_sb[32 * j:32 * (j + 1), :]
                        .rearrange("g (m n) -> g m n", n=4), src)
                scl = smp.tile([P, 16], FDT, tag="scl", name="scl")
                for m in range(M):
                    nc.vector.tensor_scalar_mul(
                        scl[:, 4 * m:4 * m + 4], s_sb[:, 4 * m:4 * m + 4],
                        conf_sb[:, bt, m:m + 1])
                ex = smp.tile([P, 16], FDT, tag="ex", name="ex")
                nc.scalar.activation(ex[:], scl[:], mybir.ActivationFunctionType.Exp)
                z4 = smp.tile([P, M], FDT, tag="z4", name="z4")
                nc.vector.reduce_sum(z4[:], ex[:].rearrange("p (m n) -> p m n", n=4),
                                     axis=AX)
                r4 = smp.tile([P, M], FDT, tag="r4", name="r4")
                nc.vector.reciprocal(r4[:], z4[:])
                w4 = smp.tile([P, M], FDT, tag="w4", name="w4")
                nc.vector.tensor_scalar_mul(w4[:], ex[:, 0:4], r4[:, 0:1])
                for m in range(1, M):
                    nc.vector.scalar_tensor_tensor(
                        w4[:], ex[:, 4 * m:4 * m + 4], r4[:, m:m + 1], w4[:],
                        op0=MUL, op1=ADD)
                w4b = smp.tile([P, M], BDT, tag="w4b", name="w4b")
                nc.vector.tensor_copy(w4b[:], w4[:])
                w_dr = drw.tile([P, M], BDT, tag="w_dr", name="w_dr")
                nc.gpsimd.dma_start(w_dr[:], w4b[:])
                wrep = wrp.tile([P, 512], BDT, tag="wrep", name="wrep")
                nc.gpsimd.dma_start(
                    wrep[:], w_dr[:].rearrange("b m -> (b m)").partition_broadcast(P))
                return wrep

            def z_combine(zT, xt, wrep, st):
                with nc.allow_low_precision(reason="4-term convex combine, fp32 acc"):
                    for hc in range(HC):
                        prod = prp.tile([P, 512], BDT, tag="prod", name="prod")
                        nc.vector.tensor_mul(
                            prod[:], xt[:, hc, st * 512:(st + 1) * 512], wrep[:])
                        nc.vector.reduce_sum(
                            zT[:, hc, st * P:(st + 1) * P],
                            prod[:].rearrange("p (b n) -> p b n", n=4), axis=AX)

            for s in range(n_super):
                T0 = s * T_SUPER
                nts = T_SUPER // 512           # 512-token slices (2)
                ntt = T_SUPER // P             # 128-token gram tiles (8)

                xt = xp.tile([P, HC, T_SUPER], BDT, tag="xt")
                for hc in range(HC):
                    nc.sync.dma_start(xt[:, hc], xT[hc * P:(hc + 1) * P,
                                                    T0:T0 + T_SUPER])
                ay = abp.tile([2, T_SUPER], BDT, tag="ay")
                nc.gpsimd.dma_start(ay[:], aby[:, T0:T0 + T_SUPER])
                ax = abp.tile([2, T_SUPER], BDT, tag="ax")
                nc.gpsimd.dma_start(ax[:], abx[:, T0:T0 + T_SUPER])

                # ---- Y = X G (feature-major: yT[h', t]) ----
                yT = yp.tile([P, OC, T_SUPER], BDT, tag="yT")
                for oc in range(OC):
                    for tsl in range(nts):
                        pt = psp.tile([P, 512], FDT, tag="proj")
                        for hc in range(HC):
                            nc.tensor.matmul(
                                pt[:], wg_sb[:, hc, oc * P:(oc + 1) * P],
                                xt[:, hc, tsl * 512:(tsl + 1) * 512],
                                start=(hc == 0), stop=(hc == HC - 1))
                        nc.scalar.copy(yT[:, oc, tsl * 512:(tsl + 1) * 512], pt[:])

                if s == 0:
                    # wc first needed by out_proj during super-tile 1; load late so
                    # these DMAs don't compete with the critical lead-in loads.
                    for oc in range(OC):
                        nc.sync.dma_start(
                            wc_sb[:, :, oc * P:(oc + 1) * P],
                            wcT[:, oc * P:(oc + 1) * P]
                            .rearrange("(c p) o -> p c o", p=P))

                # ---- gram: S_full = Y^T X + rank-1 bias fixup, per 128-token tile;
                #      two halves so each softmax can start as soon as possible ----
                gram_sb = gpl.tile([P, ntt, P], FDT, tag="gram")
                wreps = []
                for st in range(2):
                    gram_half = drg.tile([4, P, P], FDT, tag=f"gram_dr{st}",
                                         name=f"gram_dr{st}")
                    for tt in range(4 * st, 4 * st + 4):
                        gps = psg.tile([P, P], FDT, tag="gram_ps")
                        tsl = slice(tt * P, (tt + 1) * P)
                        for oc in range(OC):
                            nc.tensor.matmul(gps[:], yT[:, oc, tsl], xt[:, oc, tsl],
                                             start=(oc == 0), stop=False)
                        nc.tensor.matmul(gps[:], ay[:, tsl], ax[:, tsl],
                                         start=False, stop=True)
                        nc.scalar.copy(gram_sb[:, tt, :], gps[:])
                    nc.sync.dma_start(gram_half[:].transpose([1, 0, 2]),
                                      gram_sb[:, 4 * st:4 * st + 4, :])
                    wreps.append(softmax_block(s, st, gram_half))

                zT = zp.tile([P, HC, b_super], BDT, tag="zT")
                if s < n_super - 1:
                    if z_prev is not None:
                        out_proj(*z_prev)
                    z_combine(zT, xt, wreps[0], 0)
                    z_combine(zT, xt, wreps[1], 1)
                    z_prev = (zT, s)
                else:
                    if z_prev is not None:
                        out_proj(*z_prev)
                    z_combine(zT, xt, wreps[0], 0)
                    out_proj_st(zT, s, 0)
                    z_combine(zT, xt, wreps[1], 1)
                    out_proj_st(zT, s, 1)
    nc.compile()
    return nc


def _get_nc(n_super=T_CORE // T_SUPER):
    if n_super not in _NC_CACHE:
        _NC_CACHE[n_super] = build_bass(n_super)
    return _NC_CACHE[n_super]


def prep_in_maps(inputs, ncores=NCORES):
    """Host-side: fold weights, shard + transpose activations, cast to bf16."""
    f32 = np.float32
    f64 = np.float64
    feats = np.asarray(inputs["features"], f32)
    confs = np.asarray(inputs["confidences"], f32).reshape(-1, M)
    Wq = np.asarray(inputs["Wq"], f64)
    Wk = np.asarray(inputs["Wk"], f64)
    Wv = np.asarray(inputs["Wv"], f64)
    Wo = np.asarray(inputs["Wo"], f64)
    bq = np.asarray(inputs["bq"], f64)
    bk = np.asarray(inputs["bk"], f64)
    bv = np.asarray(inputs["bv"], f64)
    bo = np.asarray(inputs["bo"], f64)

    s = 1.0 / np.sqrt(H)
    G = (Wq * s).T @ Wk                         # [h, h']
    wg_h = np.ascontiguousarray(G).astype(BF16)
    wcT_h = np.ascontiguousarray(((Wo @ Wv) / 4.0).T).astype(BF16)
    bc_h = (bv @ Wo.T + bo).astype(f32)
    u = (Wq * s).T @ bk                         # alpha = X u + d
    w_vec = Wk.T @ (bq * s)                     # beta = X w_vec
    d = float((bq * s) @ bk)

    nb = feats.shape[0]
    b_core = nb // ncores
    X = feats.reshape(nb * M, H)
    alpha = (X @ u.astype(f32) + f32(d)).astype(f32)
    beta = (X @ w_vec.astype(f32)).astype(f32)
    t_core = b_core * M

    in_maps = []
    for c in range(ncores):
        tsl = slice(c * t_core, (c + 1) * t_core)
        xs = X[tsl]
        aby = np.ones((2, t_core), f32)
        aby[0] = alpha[tsl]
        abx = np.ones((2, t_core), f32)
        abx[1] = beta[tsl]
        in_maps.append({
            "xT": np.ascontiguousarray(xs.T).astype(BF16),
            "wg": wg_h, "wcT": wcT_h, "bc": bc_h,
            "aby": aby.astype(BF16), "abx": abx.astype(BF16),
            "conf": np.ascontiguousarray(confs[c * b_core:(c + 1) * b_core]),
        })
    return in_maps


def install_ntff_hook():
    """Best-effort shim so run_bass_kernel_spmd(trace=True) can profile under axon."""
    import types
    try:
        from antenv.axon_hooks import get_axon_ntff_profile_hook  # noqa: F401
        return True
    except ImportError:
        pass
    try:
        import antenv
        mod = types.ModuleType("antenv.axon_hooks")
        _state = {"hook": None}
        mod.set_axon_ntff_profile_hook = lambda h: _state.__setitem__("hook", h)
        mod.get_axon_ntff_profile_hook = lambda: _state["hook"]
        sys.modules["antenv.axon_hooks"] = mod
        antenv.axon_hooks = mod
        from trn_agent_boot.trn_boot import _ntff_profile_via_ctypes
        hook = _ntff_profile_via_ctypes('/opt/axon/libaxon_pjrt.so')
        if hook is None:
            return False
        mod.set_axon_ntff_profile_hook(hook)
        return True
    except Exception:
        return False


def run(inputs, trace=False, tmpdir=None):
    """Run the 8-core kernel; returns (out [B, H] f32, BassKernelResults)."""
    from concourse.bass_utils import run_bass_kernel_spmd
    nc = _get_nc()
    in_maps = prep_in_maps(inputs)
    if trace:
        install_ntff_hook()
    res = run_bass_kernel_spmd(nc, in_maps, core_ids=list(range(NCORES)),
                               trace=trace, tmpdir=tmpdir)
    out = np.concatenate(
        [np.asarray(o["outT"], np.float32).T for o in res.results], axis=0)
    return out, res


def kernel(**inputs):
    out, _ = run(inputs, trace=False)
    return out



# revision 7
# speedup vs baseline: 1.1911x; 1.1911x over previous
"""Trainium2 Bass kernel for nn_AttentionFusion (B=8192, M=4, H=1024), 8-core data parallel.

Math (exact reformulation of the reference):
  logits[b,m,n] = conf[b,m] * (y_{4b+m} . x_{4b+n}) + conf[b,m]*beta[4b+n]   (+ const_m, dropped:
                  softmax over n is shift-invariant, so the alpha/d rank-1 terms vanish)
      with Y = X G, G = (Wq/sqrt(H))^T Wk, beta = X (Wk^T bq/32)
  wt[b,n] = sum_m softmax_n(logits)[b,m,n]
  Z[b]    = sum_n wt[b,n] X[4b+n]
  out[b]  = Z[b] (Wo Wv / 4)^T  (+ bias, added on host)

v2: the score path (Y = X G and the per-group gram S = Y X^T) runs in fp8 e4m3 with
DoubleRow double-pumping (2x PE throughput); the value path stays precise: X enters the
convex combine as fp16, the combine itself is a per-partition-scalar STT on the DVE
(weights come out of softmax already in [batch-partition, 4] layout -- no DRAM broadcast),
Z is transposed to feature-major via the DMA XBAR, and the output projection is fp16.
beta*conf is precomputed on the host, killing the on-device rank-1 score fixup.
"""
import sys

if '/opt/trn_rl_repo' not in sys.path:
    sys.path.insert(0, '/opt/trn_rl_repo')

import numpy as np
import ml_dtypes

B, M, H = 8192, 4, 1024
NCORES = 8
B_CORE = B // NCORES            # 1024 batch rows per core
T_CORE = B_CORE * M             # 4096 tokens per core
T_SUPER = 512                   # tokens per super-tile (128 batch rows)
P = 128
OC = H // P                     # 8 output chunks
HC = H // P                     # 8 contraction chunks
F8 = ml_dtypes.float8_e4m3      # TRN e4m3: max normal 240
F16 = np.float16

_NC_CACHE = {}


def build_bass(n_super=T_CORE // T_SUPER):
    import concourse.bass as bass
    import concourse.mybir as mybir
    import concourse.tile as tile
    from concourse import bacc

    t_core = n_super * T_SUPER
    b_core = t_core // M
    b_super = T_SUPER // M                 # 128 batch rows per super-tile

    nc = bacc.Bacc(None, target_bir_lowering=False)
    xT = nc.dram_tensor("xT", [H, t_core], mybir.dt.float8e4, kind="ExternalInput")
    xg = nc.dram_tensor("xg", [b_core, M * H], mybir.dt.float16, kind="ExternalInput")
    wg = nc.dram_tensor("wg", [H, H], mybir.dt.float8e4, kind="ExternalInput")
    wcT = nc.dram_tensor("wcT", [H, H], mybir.dt.float16, kind="ExternalInput")
    conf = nc.dram_tensor("conf", [b_core, M], mybir.dt.float32, kind="ExternalInput")
    cb16 = nc.dram_tensor("cb16", [b_core, 16], mybir.dt.float32, kind="ExternalInput")
    syv = nc.dram_tensor("syv", [1], mybir.dt.float32, kind="ExternalInput")
    outT = nc.dram_tensor("outT", [H, b_core], mybir.dt.float16, kind="ExternalOutput")

    FDT = mybir.dt.float32
    DT16 = mybir.dt.float16
    DT8 = mybir.dt.float8e4
    BDT = mybir.dt.bfloat16
    AX = mybir.AxisListType.X
    MUL = mybir.AluOpType.mult
    ADD = mybir.AluOpType.add
    DR = mybir.MatmulPerfMode.DoubleRow
    COPY = mybir.ActivationFunctionType.Copy
    EXP = mybir.ActivationFunctionType.Exp

    from contextlib import ExitStack
    with tile.TileContext(nc) as tc:
        with ExitStack() as _es:
            wp = _es.enter_context(tc.tile_pool(name="wp", bufs=1))
            cp = _es.enter_context(tc.tile_pool(name="cp", bufs=1))
            xp = _es.enter_context(tc.tile_pool(name="xp", bufs=3))
            xgp = _es.enter_context(tc.tile_pool(name="xgp", bufs=3))
            yp = _es.enter_context(tc.tile_pool(name="yp", bufs=3))
            gpl = _es.enter_context(tc.tile_pool(name="gp", bufs=2))
            smp = _es.enter_context(tc.tile_pool(name="smp", bufs=2))
            zp = _es.enter_context(tc.tile_pool(name="zp", bufs=2))
            ztp = _es.enter_context(tc.tile_pool(name="ztp", bufs=2))
            osb = _es.enter_context(tc.tile_pool(name="osb", bufs=2))
            psp = _es.enter_context(tc.tile_pool(name="psp", bufs=3, space="PSUM"))
            psg = _es.enter_context(tc.tile_pool(name="psg", bufs=2, space="PSUM"))
            pso = _es.enter_context(tc.tile_pool(name="pso", bufs=3, space="PSUM"))
            drg = _es.enter_context(tc.tile_pool(name="drg", bufs=n_super, space="DRAM"))

            # ---- HAM warmup: dataless matmuls spin the PE to the warm clock
            #      while the lead-in DMAs stream ----
            wu = wp.tile([P, P], BDT, tag="warm", name="warm_sb")
            nc.vector.memset(wu[:], 1.0)
            wups = psg.tile([P, P], FDT, tag="gram_ps", name="warm_ps")
            for i in range(40):
                nc.tensor.matmul(wups[:], wu[:], wu[:],
                                 start=(i == 0), stop=(i == 39))
            wuo = wp.tile([P, P], FDT, tag="warmo", name="warm_out")
            nc.scalar.copy(wuo[:], wups[:])

            # ---- resident weights / constants (wg chunked by output cols so the
            #      first Y matmul only waits on chunk 0) ----
            wg_sb = wp.tile([P, HC, H], DT8, tag="wg", name="wg_sb")
            for oc in range(OC):
                nc.sync.dma_start(
                    wg_sb[:, :, oc * P:(oc + 1) * P],
                    wg[:, oc * P:(oc + 1) * P].rearrange("(c p) o -> p c o", p=P))
            conf_sb = cp.tile([P, n_super, M], FDT, tag="conf", name="conf_sb")
            nc.sync.dma_start(conf_sb[:], conf[:].rearrange("(bt p) m -> p bt m", p=P))
            cb_sb = cp.tile([P, n_super, 16], FDT, tag="cb", name="cb_sb")
            nc.sync.dma_start(cb_sb[:], cb16[:].rearrange("(bt p) q -> p bt q", p=P))
            sy_sb = cp.tile([P, 1], FDT, tag="sy", name="sy_sb")
            nc.sync.dma_start(sy_sb[:], syv[:].partition_broadcast(P))
            wc_sb = wp.tile([P, HC, H], DT16, tag="wc", name="wc_sb")

            def y_proj(s, xt):
                """Y = X G in fp8 DoubleRow; quantize back to fp8 with scale sy."""
                T0 = s * T_SUPER
                yT = yp.tile([P, OC, T_SUPER], DT8, tag="yT")
                for oc in range(OC):
                    pt = psp.tile([P, T_SUPER], FDT, tag="proj")
                    for kk in range(HC // 2):
                        nc.tensor.matmul(
                            pt[:], wg_sb[:, 2 * kk:2 * kk + 2, oc * P:(oc + 1) * P],
                            xt[:, 2 * kk:2 * kk + 2, :],
                            start=(kk == 0), stop=(kk == HC // 2 - 1),
                            perf_mode=DR)
                    nc.scalar.activation(yT[:, oc, :], pt[:], COPY,
                                         scale=sy_sb[:])
                return yT

            def gram_softmax(s, xt, yT, xgt):
                """Block-diag scores -> softmax -> STT combine -> XBAR transpose."""
                gram_sb = gpl.tile([P, 4, P], FDT, tag="gram")
                for tt in range(4):
                    gps = psg.tile([P, P], FDT, tag="gram_ps")
                    tsl = slice(tt * P, (tt + 1) * P)
                    for kk in range(HC // 2):
                        nc.tensor.matmul(
                            gps[:], yT[:, 2 * kk:2 * kk + 2, tsl],
                            xt[:, 2 * kk:2 * kk + 2, tsl],
                            start=(kk == 0), stop=(kk == HC // 2 - 1),
                            perf_mode=DR)
                    nc.vector.tensor_copy(gram_sb[:, tt, :], gps[:])
                gram_dr = drg.tile([4, P, P], FDT, tag="gram_dr", name=f"gram_dr{s}")
                nc.sync.dma_start(gram_dr[:].transpose([1, 0, 2]), gram_sb[:])
                s_sb = smp.tile([P, 16], FDT, tag="s", name="s_sb")
                for j in range(4):
                    base = gram_dr[:]
                    src = bass.AP(base.tensor, base.offset + j * P * P,
                                  [[4 * P + 4, 32], [P, 4], [1, 4]])
                    nc.sync.dma_start(
                        s_sb[32 * j:32 * (j + 1), :]
                        .rearrange("g (m n) -> g m n", n=4), src)
                # logits = s_raw*conf_dev + conf*beta
                scl = smp.tile([P, 16], FDT, tag="scl", name="scl")
                for m in range(M):
                    nc.vector.scalar_tensor_tensor(
                        scl[:, 4 * m:4 * m + 4], s_sb[:, 4 * m:4 * m + 4],
                        conf_sb[:, s, m:m + 1], cb_sb[:, s, 4 * m:4 * m + 4],
                        op0=MUL, op1=ADD)
                ex = smp.tile([P, 16], FDT, tag="ex", name="ex")
                nc.scalar.activation(ex[:], scl[:], EXP)
                z4 = smp.tile([P, M], FDT, tag="z4", name="z4")
                nc.vector.reduce_sum(z4[:], ex[:].rearrange("p (m n) -> p m n", n=4),
                                     axis=AX)
                r4 = smp.tile([P, M], FDT, tag="r4", name="r4")
                nc.vector.reciprocal(r4[:], z4[:])
                w4 = smp.tile([P, M], FDT, tag="w4", name="w4")
                nc.vector.tensor_scalar_mul(w4[:], ex[:, 0:4], r4[:, 0:1])
                for m in range(1, M):
                    nc.vector.scalar_tensor_tensor(
                        w4[:], ex[:, 4 * m:4 * m + 4], r4[:, m:m + 1], w4[:],
                        op0=MUL, op1=ADD)
                # convex combine in batch-major layout: Z[g, h] = sum_n w4[g,n] X[4g+n, h]
                with nc.allow_low_precision(reason="convex combine, fp32 acc"):
                    acc = smp.tile([P, H], FDT, tag="acc", name="acc")
                    nc.vector.tensor_scalar_mul(acc[:], xgt[:, 0:H], w4[:, 0:1])
                    for n in range(1, M - 1):
                        nc.vector.scalar_tensor_tensor(
                            acc[:], xgt[:, n * H:(n + 1) * H], w4[:, n:n + 1],
                            acc[:], op0=MUL, op1=ADD)
                    zb = zp.tile([P, H], DT16, tag="zb")
                    nc.vector.scalar_tensor_tensor(
                        zb[:], xgt[:, (M - 1) * H:M * H], w4[:, M - 1:M], acc[:],
                        op0=MUL, op1=ADD)
                zT = ztp.tile([P, HC, P], DT16, tag="zT")
                nc.sync.dma_start_transpose(zT[:], zb[:])
                return zT

            def out_proj(s, zT):
                o_sb = osb.tile([P, OC, P], DT16, tag="osb")
                for oc in range(OC):
                    po = pso.tile([P, P], FDT, tag="outp")
                    for hc in range(HC):
                        nc.tensor.matmul(
                            po[:], wc_sb[:, hc, oc * P:(oc + 1) * P], zT[:, hc, :],
                            start=(hc == 0), stop=(hc == HC - 1))
                    if oc % 2 == 0:
                        nc.scalar.copy(o_sb[:, oc, :], po[:])
                    else:
                        nc.vector.tensor_copy(o_sb[:, oc, :], po[:])
                nc.sync.dma_start(
                    outT[:, s * b_super:(s + 1) * b_super]
                    .rearrange("(c p) b -> p c b", p=P), o_sb[:])

            state = {}  # s -> (zT,) pending; gram deferred one super-tile
            prev = {}
            for s in range(n_super):
                T0 = s * T_SUPER
                xt = xp.tile([P, HC, T_SUPER], DT8, tag="xt")
                for hc in range(HC):
                    nc.sync.dma_start(xt[:, hc], xT[hc * P:(hc + 1) * P,
                                                    T0:T0 + T_SUPER])
                xgt = xgp.tile([P, M * H], DT16, tag="xg")
                nc.sync.dma_start(xgt[:], xg[s * b_super:(s + 1) * b_super, :])

                yT = y_proj(s, xt)
                if s == 0:
                    # wc first needed by out_proj(0) two super-tiles from now
                    for oc in range(OC):
                        nc.sync.dma_start(
                            wc_sb[:, :, oc * P:(oc + 1) * P],
                            wcT[:, oc * P:(oc + 1) * P]
                            .rearrange("(c p) o -> p c o", p=P))
                state[s] = (xt, yT, xgt)
                if s >= 1:
                    prev[s - 1] = gram_softmax(s - 1, *state.pop(s - 1))
                if s >= 2:
                    out_proj(s - 2, prev.pop(s - 2))
            sl = n_super - 1
            prev[sl] = gram_softmax(sl, *state.pop(sl))
            out_proj(sl - 1, prev.pop(sl - 1))
            out_proj(sl, prev.pop(sl))
    nc.compile()
    return nc


def _get_nc(n_super=T_CORE // T_SUPER):
    if n_super not in _NC_CACHE:
        _NC_CACHE[n_super] = build_bass(n_super)
    return _NC_CACHE[n_super]


def prep_in_maps(inputs, ncores=NCORES):
    """Host-side: fold weights, pick fp8 scales, shard + transpose, cast."""
    f32 = np.float32
    f64 = np.float64
    feats = np.asarray(inputs["features"], f32)
    confs = np.asarray(inputs["confidences"], f32).reshape(-1, M)
    Wq = np.asarray(inputs["Wq"], f64)
    Wk = np.asarray(inputs["Wk"], f64)
    Wv = np.asarray(inputs["Wv"], f64)
    Wo = np.asarray(inputs["Wo"], f64)
    bq = np.asarray(inputs["bq"], f64)
    bv = np.asarray(inputs["bv"], f64)
    bo = np.asarray(inputs["bo"], f64)

    s = 1.0 / np.sqrt(H)
    G = (Wq * s).T @ Wk                         # [h, h']
    wcT_h = np.ascontiguousarray(((Wo @ Wv) / 4.0).T).astype(F16)
    bc_h = (bv @ Wo.T + bo).astype(f32)         # added on host after the run
    w_vec = Wk.T @ (bq * s)                     # beta = X w_vec

    nb = feats.shape[0]
    b_core = nb // ncores
    t_core = b_core * M
    X = feats.reshape(nb * M, H)

    # fp8 scales: X and G use absmax -> 224; Y's scale comes from G column norms
    sx = f32(224.0 / np.abs(X).max())
    sg = f32(224.0 / np.abs(G).max())
    XT8 = np.ascontiguousarray((X * sx).T).astype(F8)
    wg_h = np.ascontiguousarray(G * sg).astype(F8)
    col_sig = np.sqrt((np.asarray(wg_h, f32) ** 2).sum(axis=0)).max() * sx
    sy = f32(224.0 / (6.5 * col_sig))           # fp8-Y sigma ~34, 6.5-sigma headroom
    descale = f32(1.0) / (f32(sx) * f32(sx) * f32(sg) * f32(sy))

    beta = (X @ w_vec.astype(f32)).astype(f32)  # [nb*M]
    conf_dev = confs * descale
    cbeta = confs[:, :, None] * beta.reshape(nb, M)[:, None, :]   # [b, m, n]
    cb16_h = np.ascontiguousarray(cbeta.reshape(nb, 16)).astype(f32)
    xg_h = feats.reshape(nb, M * H).astype(F16)

    in_maps = []
    for c in range(ncores):
        tsl = slice(c * t_core, (c + 1) * t_core)
        bsl = slice(c * b_core, (c + 1) * b_core)
        in_maps.append({
            "xT": np.ascontiguousarray(XT8[:, tsl]),
            "xg": np.ascontiguousarray(xg_h[bsl]),
            "wg": wg_h, "wcT": wcT_h,
            "conf": np.ascontiguousarray(conf_dev[bsl]),
            "cb16": np.ascontiguousarray(cb16_h[bsl]),
            "syv": np.array([sy], f32),
        })
    return in_maps, bc_h


def install_ntff_hook():
    """Best-effort shim so run_bass_kernel_spmd(trace=True) can profile under axon."""
    import types
    try:
        from antenv.axon_hooks import get_axon_ntff_profile_hook  # noqa: F401
        return True
    except ImportError:
        pass
    try:
        import antenv
        mod = types.ModuleType("antenv.axon_hooks")
        _state = {"hook": None}
        mod.set_axon_ntff_profile_hook = lambda h: _state.__setitem__("hook", h)
        mod.get_axon_ntff_profile_hook = lambda: _state["hook"]
        sys.modules["antenv.axon_hooks"] = mod
        antenv.axon_hooks = mod
        from trn_agent_boot.trn_boot import _ntff_profile_via_ctypes
        hook = _ntff_profile_via_ctypes('/opt/axon/libaxon_pjrt.so')
        if hook is None:
            return False
        mod.set_axon_ntff_profile_hook(hook)
        return True
    except Exception:
        return False


def run(inputs, trace=False, tmpdir=None):
    """Run the 8-core kernel; returns (out [B, H] f32, BassKernelResults)."""
    from concourse.bass_utils import run_bass_kernel_spmd
    nc = _get_nc()
    in_maps, bc_h = prep_in_maps(inputs)
    if trace:
        install_ntff_hook()
    res = run_bass_kernel_spmd(nc, in_maps, core_ids=list(range(NCORES)),
                               trace=trace, tmpdir=tmpdir)
    out = np.concatenate(
        [np.asarray(o["outT"], np.float32).T for o in res.results], axis=0)
    out += bc_h[None, :]
    return out, res


def kernel(**inputs):
    out, _ = run(inputs, trace=False)
    return out
